# revision 12
# baseline (speedup 1.0000x reference)
"""Trainium2 Bass kernel for dynamic k-max pooling (per-column top-k with
order-preserving compaction), data-parallel over batch across 8 NeuronCores.

Self-contained: hardcodes shapes B=16, T=4096, C=256. Host does only O(B)
scalar prep (per-sample k, Newton slope, tail mask rows); all O(B*T*C) work
runs on-device:
  - DMA + PE-transpose x[s] into [C, T] layout; invalid tail rows masked to
    -1e30 during PSUM evacuation
  - per-column exact threshold t* (k-th largest): 4 damped Newton iterations
    on exact is_ge counts, then exact order-statistic extraction through a
    two-sided top-8 window (nc.vector.max on masked/negated streams)
  - output positions via custom-DVE prefix-scan ops
  - order-preserving compaction via gpsimd local_scatter of the two 16-bit
    halves of each f32, then a single shift-or recombine
  - PE-transpose back to [T, C]; DMA out
"""

import numpy as np

import concourse.bass as bass
import concourse.mybir as mybir
import concourse.tile as tile
from concourse import bacc
import concourse.dve_ops as dve_ops_mod
from concourse.dve_ops import DveOp
from concourse.dve_spec import (
    Spec, Src0, C0, C1, C2, Zero, One, AluOp,
    scan, select, lower, _has_src1 as has_src1,
)
from concourse.dve_uop import DveOpSpec
from concourse.bass_utils import run_bass_kernel_spmd

F32 = mybir.dt.float32
U32 = mybir.dt.uint32
I16 = mybir.dt.int16
U16 = mybir.dt.uint16
BF16 = mybir.dt.bfloat16
Alu = mybir.AluOpType
Act = mybir.ActivationFunctionType

B, T, C = 16, 4096, 256
NCORES = 8
SPC = B // NCORES          # samples per core
NEG = np.float32(-1e30)
TH0 = -0.6745              # initial threshold guess (25% drop quantile)
PHI0 = 0.31777657          # N(0,1) pdf at TH0
DAMP = 0.6
SC = float(2.0 ** -20)     # exact pow2 scale for the masked-neg pass
CHA = 2046                 # scatter chunk A covers pos [0, 2046)
CHB = 1026                 # chunk B covers pos [2046, 3072)
WA = 3070                  # chunk A source window [0, 3070)  (2046 + 1024)
MAXDROP = 1024


# ---------------- custom DVE ops ----------------------------------------- #

def _register(name, spec, subdim=False):
    if name in dve_ops_mod._SUB_OPCODE_FOR_NAME:
        return next(op for op in dve_ops_mod.OPS if op.name == name)
    row = dve_ops_mod._CUSTOM_DVE_ROW_BASE + len(dve_ops_mod.OPS)
    assert row < 0x20
    shas = {}
    for ver in ("v3", "v4"):
        uops = lower(spec, ver=ver)
        tmp = DveOpSpec(name=name, opcode=row, uops=uops, rd1_en=has_src1(spec))
        shas[ver] = tmp.sha(ver)
    op = DveOp(name, spec, subdim=subdim, uops_sha=shas)
    dve_ops_mod.OPS.append(op)
    dve_ops_mod._SUB_OPCODE_FOR_NAME[name] = row
    dve_ops_mod.CUSTOM_DVE_SPECS[name] = spec
    return op


def _ref_with_accum(body):
    def r(in0, s0, s1, imm2):
        o = body(in0, s0, s1, imm2)
        return o, o.reshape(o.shape[0], -1).sum(axis=-1, keepdims=True)
    return r


# wa = (v > t) ? -v*imm2 : -1 ; accum = sum(wa)  -> exact count above
MNSA = _register("DKP_MNSA", Spec(
    body=select(Src0 > C0, (Zero - Src0) * C2, Zero - One),
    accum=AluOp.ADD,
    reference=_ref_with_accum(lambda in0, s0, s1, imm2: np.where(
        in0 > s0, -in0 * np.float32(imm2), np.float32(-1.0)).astype(np.float32)),
))

# wb = (v > t) ? -1 : v*imm2   (top8 -> 8 largest below-or-equal t, scaled)
MNSB = _register("DKP_MNSB", Spec(
    body=select(Src0 > C0, Zero - One, Src0 * C2),
    reference=lambda in0, s0, s1, imm2: np.where(
        in0 > s0, np.float32(-1.0), in0 * np.float32(imm2)).astype(np.float32),
))

# posA: o = v >= t ; p = scan(+, o, init=-1); out = (o & (p < s1)) ? p : -1
_o = Src0 >= C0
_p = scan(AluOp.ADD, _o, init=Zero - One)
POSA = _register("DKP_POSA", Spec(
    body=select(_o & (_p < C1), _p, Zero - One),
    reference=lambda in0, s0, s1, imm2: (lambda o, p: np.where(
        o & (p < s1), p, np.float32(-1.0)))(
        in0 >= s0,
        np.cumsum(in0 >= s0, axis=-1, dtype=np.float32) - 1.0
    ).astype(np.float32),
))

# posB: o = v >= t ; p = scan(+, o, init=s1); out = (o & (p < imm2)) ? p : -1
_ob = Src0 >= C0
_pb = scan(AluOp.ADD, _ob, init=C1)
POSB = _register("DKP_POSB", Spec(
    body=select(_ob & (_pb < C2), _pb, Zero - One),
    reference=lambda in0, s0, s1, imm2: (lambda o, p: np.where(
        o & (p < imm2), p, np.float32(-1.0)))(
        in0 >= s0,
        np.cumsum(in0 >= s0, axis=-1, dtype=np.float32) + s1
    ).astype(np.float32),
))


# ---------------- device program ----------------------------------------- #

def build_program():
    import os
    STAGE = int(os.environ.get("DKP_STAGE", "6"))
    nc = bacc.Bacc()
    x2 = nc.declare_dram_parameter("x2", [SPC, T, C], F32, isOutput=False)
    mrowb = nc.declare_dram_parameter("mrowb", [SPC, 128, 2048], F32, isOutput=False)
    kf_d = nc.declare_dram_parameter("kf", [SPC, 128, 1], F32, isOutput=False)
    isl_d = nc.declare_dram_parameter("isl", [SPC, 128, 1], F32, isOutput=False)
    iota8_d = nc.declare_dram_parameter("iota8", [128, 8], F32, isOutput=False)
    ident_d = nc.declare_dram_parameter("ident", [128, 128], F32, isOutput=False)
    y2 = nc.declare_dram_parameter("y2", [SPC, T, C], F32, isOutput=True)

    with tile.TileContext(nc) as tc:
        with (
            tc.tile_pool(name="nat", bufs=1) as nat_pool,
            tc.tile_pool(name="mrow", bufs=2) as mrow_pool,
            tc.tile_pool(name="xt", bufs=2) as xt_pool,
            tc.tile_pool(name="scr", bufs=1) as scr_pool,
            tc.tile_pool(name="w", bufs=1) as w_pool,
            tc.tile_pool(name="pos", bufs=1) as pos_pool,
            tc.tile_pool(name="pl", bufs=1) as pl_pool,
            tc.tile_pool(name="dst", bufs=2) as dst_pool,
            tc.tile_pool(name="ot", bufs=1) as ot_pool,
            tc.tile_pool(name="onat", bufs=1) as onat_pool,
            tc.tile_pool(name="small", bufs=4) as sm_pool,
            tc.tile_pool(name="cst", bufs=1) as cst_pool,
            tc.tile_pool(name="ps", bufs=4, space="PSUM") as ps_pool,
        ):
            ident = cst_pool.tile([128, 128], F32, tag="ident")
            nc.sync.dma_start(ident[:], ident_d[:])
            iota8 = cst_pool.tile([128, 8], F32, tag="iota8")
            nc.sync.dma_start(iota8[:], iota8_d[:])

            for s in range(SPC):
                mrow_t = mrow_pool.tile([128, 2048], F32, tag="mrow")
                nc.sync.dma_start(mrow_t[:], mrowb[s])
                kf_t = sm_pool.tile([128, 1], F32, tag="kf")
                nc.sync.dma_start(kf_t[:], kf_d[s])
                isl_t = sm_pool.tile([128, 1], F32, tag="isl")
                nc.sync.dma_start(isl_t[:], isl_d[s])

                for ct in range(2):
                    # ---- load natural layout [128 rows, 32 blocks, 128 cols]
                    nat = nat_pool.tile([128, 32, 128], F32, tag="nat")
                    src = x2[s].rearrange("(b p) c -> p b c", p=128)
                    nc.sync.dma_start(nat[:], src[:, :, ct * 128:(ct + 1) * 128])

                    # ---- transpose to xT [128 cols, 4096 rows], mask tail
                    xt = xt_pool.tile([128, T], F32, tag="xt")
                    for g in range(8):
                        ps = ps_pool.tile([128, 512], F32, tag="psA")
                        for j in range(4):
                            nc.tensor.transpose(
                                ps[:, j * 128:(j + 1) * 128],
                                nat[:, 4 * g + j, :], ident[:],
                            )
                        if g < 4:
                            nc.scalar.activation(
                                xt[:, g * 512:(g + 1) * 512], ps[:], Act.Copy)
                        else:
                            nc.vector.tensor_tensor(
                                xt[:, g * 512:(g + 1) * 512], ps[:],
                                mrow_t[:, (g - 4) * 512:(g - 3) * 512],
                                Alu.add)

                    # ---- Newton iterations for threshold
                    if STAGE < 2:
                        onat = onat_pool.tile([128, 32, 128], F32, tag="onat")
                        for g in range(8):
                            ps2 = ps_pool.tile([128, 512], F32, tag="psB")
                            for j in range(4):
                                blk = 4 * g + j
                                nc.tensor.transpose(
                                    ps2[:, j * 128:(j + 1) * 128],
                                    xt[:, blk * 128:(blk + 1) * 128], ident[:])
                            nc.scalar.activation(
                                onat[:, 4 * g:4 * g + 4, :].rearrange(
                                    "p b c -> p (b c)"),
                                ps2[:], Act.Copy)
                        dstv = y2[s].rearrange("(b p) c -> p b c", p=128)
                        nc.sync.dma_start(
                            dstv[:, :, ct * 128:(ct + 1) * 128], onat[:])
                        continue
                    th = sm_pool.tile([128, 1], F32, tag="th")
                    nc.vector.memset(th[:], TH0)
                    scr = scr_pool.tile([128, T], BF16, tag="scr")
                    nt = sm_pool.tile([128, 1], F32, tag="nt")
                    d1 = sm_pool.tile([128, 1], F32, tag="d1")
                    for it in range(4):
                        nc.vector.tensor_scalar(
                            scr[:], xt[:], th[:], None, Alu.is_ge,
                            Alu.add, accum_out=nt[:])
                        nc.vector.scalar_tensor_tensor(
                            d1[:], nt[:], kf_t[:], isl_t[:],
                            Alu.subtract, Alu.mult)
                        nc.vector.tensor_tensor(th[:], th[:], d1[:], Alu.add)

                    # ---- two-sided order-statistic window
                    if STAGE < 3:
                        ot = ot_pool.tile([128, T], F32, tag="ot")
                        nc.vector.tensor_scalar(ot[:], xt[:], th[:], None, Alu.subtract)
                        onat = onat_pool.tile([128, 32, 128], F32, tag="onat")
                        for g in range(8):
                            ps2 = ps_pool.tile([128, 512], F32, tag="psB")
                            for j in range(4):
                                blk = 4 * g + j
                                nc.tensor.transpose(
                                    ps2[:, j * 128:(j + 1) * 128],
                                    ot[:, blk * 128:(blk + 1) * 128], ident[:])
                            nc.scalar.activation(
                                onat[:, 4 * g:4 * g + 4, :].rearrange(
                                    "p b c -> p (b c)"),
                                ps2[:], Act.Copy)
                        dstv = y2[s].rearrange("(b p) c -> p b c", p=128)
                        nc.sync.dma_start(
                            dstv[:, :, ct * 128:(ct + 1) * 128], onat[:])
                        continue
                    wa = w_pool.tile([128, T], F32, tag="w")
                    swa = sm_pool.tile([128, 1], F32, tag="swa")
                    nc.vector._custom_dve(
                        MNSA, out=wa[:], in0=xt[:], s0=th[:], imm2=SC,
                        accum_out=swa[:])
                    wa8 = sm_pool.tile([128, 8], F32, tag="wa8")
                    nc.vector.max(wa8[:], wa[:])
                    wb = w_pool.tile([128, T], F32, tag="w")
                    nc.vector._custom_dve(MNSB, out=wb[:], in0=xt[:], s0=th[:],
                                          imm2=SC)
                    wb8 = sm_pool.tile([128, 8], F32, tag="wb8")
                    nc.vector.max(wb8[:], wb[:])

                    # ---- t* = selected order statistic
                    # ma = (T - k) + swa   (= m up to frac error in (-0.024, 0))
                    ma = sm_pool.tile([128, 1], F32, tag="ma")
                    nc.vector.tensor_scalar(
                        ma[:], kf_t[:], -1.0, float(T), Alu.mult, Alu.add)
                    nc.vector.tensor_tensor(ma[:], ma[:], swa[:], Alu.add)
                    da = sm_pool.tile([128, 8], F32, tag="da")
                    nc.vector.tensor_scalar(da[:], iota8[:], ma[:], None,
                                            Alu.subtract)
                    mka = sm_pool.tile([128, 8], F32, tag="mka")
                    nc.vector.tensor_tensor(mka[:], da[:], da[:], Alu.mult)
                    nc.vector.tensor_scalar(mka[:], mka[:], 0.2, None, Alu.is_lt)
                    ta = sm_pool.tile([128, 1], F32, tag="ta")
                    junk8 = sm_pool.tile([128, 8], F32, tag="junk8")
                    nc.vector.tensor_tensor(junk8[:], mka[:], wa8[:], Alu.mult)
                    nc.vector.tensor_reduce(
                        ta[:], junk8[:], mybir.AxisListType.X, Alu.add)
                    # mb = -ma - 1
                    mb = sm_pool.tile([128, 1], F32, tag="mb")
                    nc.vector.tensor_scalar(mb[:], ma[:], -1.0, -1.0,
                                            Alu.mult, Alu.add)
                    db = sm_pool.tile([128, 8], F32, tag="db")
                    nc.vector.tensor_scalar(db[:], iota8[:], mb[:], None,
                                            Alu.subtract)
                    mkb = sm_pool.tile([128, 8], F32, tag="mkb")
                    nc.vector.tensor_tensor(mkb[:], db[:], db[:], Alu.mult)
                    nc.vector.tensor_scalar(mkb[:], mkb[:], 0.2, None, Alu.is_lt)
                    tb = sm_pool.tile([128, 1], F32, tag="tb")
                    junk8b = sm_pool.tile([128, 8], F32, tag="junk8b")
                    nc.vector.tensor_tensor(junk8b[:], mkb[:], wb8[:], Alu.mult)
                    nc.vector.tensor_reduce(
                        tb[:], junk8b[:], mybir.AxisListType.X, Alu.add)
                    tstar = sm_pool.tile([128, 1], F32, tag="tstar")
                    nc.vector.tensor_tensor(tstar[:], tb[:], ta[:], Alu.subtract)
                    nc.vector.tensor_scalar(tstar[:], tstar[:],
                                            float(2.0 ** 20), None, Alu.mult)

                    # ---- positions
                    if STAGE < 4:
                        ot = ot_pool.tile([128, T], F32, tag="ot")
                        nc.vector.tensor_scalar(ot[:], xt[:], tstar[:], None, Alu.is_ge)
                        onat = onat_pool.tile([128, 32, 128], F32, tag="onat")
                        for g in range(8):
                            ps2 = ps_pool.tile([128, 512], F32, tag="psB")
                            for j in range(4):
                                blk = 4 * g + j
                                nc.tensor.transpose(
                                    ps2[:, j * 128:(j + 1) * 128],
                                    ot[:, blk * 128:(blk + 1) * 128], ident[:])
                            nc.scalar.activation(
                                onat[:, 4 * g:4 * g + 4, :].rearrange(
                                    "p b c -> p (b c)"),
                                ps2[:], Act.Copy)
                        dstv = y2[s].rearrange("(b p) c -> p b c", p=128)
                        nc.sync.dma_start(
                            dstv[:, :, ct * 128:(ct + 1) * 128], onat[:])
                        continue
                    nbf = sm_pool.tile([128, 1], F32, tag="nbf")
                    nc.vector.tensor_scalar(
                        scr[:, :CHA], xt[:, :CHA], tstar[:], None, Alu.is_ge,
                        Alu.add, accum_out=nbf[:])
                    pbinit = sm_pool.tile([128, 1], F32, tag="pbinit")
                    nc.vector.tensor_scalar(pbinit[:], nbf[:],
                                            float(-1 - CHA), None, Alu.add)
                    posa = pos_pool.tile([128, WA], I16, tag="posa")
                    nc.vector._custom_dve(
                        POSA, out=posa[:], in0=xt[:, :WA], s0=tstar[:],
                        s1=float(CHA))
                    posb = pos_pool.tile([128, T - CHA], I16, tag="posb")
                    nc.vector._custom_dve(
                        POSB, out=posb[:], in0=xt[:, CHA:], s0=tstar[:],
                        s1=pbinit[:], imm2=float(CHB))

                    # ---- 16-bit planes (little-endian halves, strided copies)
                    xt_h = xt[:].bitcast(U16).rearrange("p (t two) -> p t two", two=2)
                    hi16 = pl_pool.tile([128, T], U16, tag="hi16")
                    nc.vector.tensor_copy(hi16[:], xt_h[:, :, 1])
                    lo16 = pl_pool.tile([128, T], U16, tag="lo16")
                    nc.vector.tensor_copy(lo16[:], xt_h[:, :, 0])

                    # ---- scatters
                    if STAGE < 5:
                        ot = ot_pool.tile([128, T], F32, tag="ot")
                        nc.vector.tensor_copy(ot[:, :WA], posa[:])
                        nc.vector.tensor_copy(ot[:, WA:WA + (T - CHA)], posb[:])
                        nc.vector.memset(ot[:, WA + T - CHA:], 0.0)
                        onat = onat_pool.tile([128, 32, 128], F32, tag="onat")
                        for g in range(8):
                            ps2 = ps_pool.tile([128, 512], F32, tag="psB")
                            for j in range(4):
                                blk = 4 * g + j
                                nc.tensor.transpose(
                                    ps2[:, j * 128:(j + 1) * 128],
                                    ot[:, blk * 128:(blk + 1) * 128], ident[:])
                            nc.scalar.activation(
                                onat[:, 4 * g:4 * g + 4, :].rearrange(
                                    "p b c -> p (b c)"),
                                ps2[:], Act.Copy)
                        dstv = y2[s].rearrange("(b p) c -> p b c", p=128)
                        nc.sync.dma_start(
                            dstv[:, :, ct * 128:(ct + 1) * 128], onat[:])
                        continue
                    dstA_lo = dst_pool.tile([128, CHA], U16, tag="dal")
                    dstA_hi = dst_pool.tile([128, CHA], U16, tag="dah")
                    dstB_lo = dst_pool.tile([128, CHB], U16, tag="dbl")
                    dstB_hi = dst_pool.tile([128, CHB], U16, tag="dbh")
                    nc.gpsimd.local_scatter(
                        dstA_lo[:], lo16[:, :WA], posa[:],
                        channels=128, num_elems=CHA, num_idxs=WA)
                    nc.gpsimd.local_scatter(
                        dstA_hi[:], hi16[:, :WA], posa[:],
                        channels=128, num_elems=CHA, num_idxs=WA)
                    nc.gpsimd.local_scatter(
                        dstB_lo[:], lo16[:, CHA:], posb[:],
                        channels=128, num_elems=CHB, num_idxs=T - CHA)
                    nc.gpsimd.local_scatter(
                        dstB_hi[:], hi16[:, CHA:], posb[:],
                        channels=128, num_elems=CHB, num_idxs=T - CHA)

                    # ---- recombine into outT f32 [128, 4096] (interleaving copies)
                    ot = ot_pool.tile([128, T], F32, tag="ot")
                    ot_h = ot[:].bitcast(U16).rearrange("p (t two) -> p t two", two=2)
                    nc.vector.tensor_copy(ot_h[:, :CHA, 0], dstA_lo[:])
                    nc.vector.tensor_copy(ot_h[:, :CHA, 1], dstA_hi[:])
                    nc.vector.tensor_copy(ot_h[:, CHA:CHA + CHB, 0], dstB_lo[:])
                    nc.vector.tensor_copy(ot_h[:, CHA:CHA + CHB, 1], dstB_hi[:])
                    nc.vector.memset(ot[:, CHA + CHB:], 0.0)

                    # ---- transpose back + store
                    onat = onat_pool.tile([128, 32, 128], F32, tag="onat")
                    for g in range(8):
                        ps2 = ps_pool.tile([128, 512], F32, tag="psB")
                        for j in range(4):
                            blk = 4 * g + j
                            nc.tensor.transpose(
                                ps2[:, j * 128:(j + 1) * 128],
                                ot[:, blk * 128:(blk + 1) * 128], ident[:],
                            )
                        nc.scalar.activation(
                            onat[:, 4 * g:4 * g + 4, :].rearrange(
                                "p b c -> p (b c)"),
                            ps2[:], Act.Copy)
                    dstv = y2[s].rearrange("(b p) c -> p b c", p=128)
                    nc.sync.dma_start(
                        dstv[:, :, ct * 128:(ct + 1) * 128], onat[:])
    return nc


_PROGRAM = None


def _get_program():
    global _PROGRAM
    if _PROGRAM is None:
        nc = build_program()
        if not nc.is_finalized():
            nc.finalize()
        _PROGRAM = nc
    return _PROGRAM


def kernel(x, lengths, pool_ranges, top_k, layer, total_layers):
    x = np.asarray(x, dtype=np.float32)
    lengths = np.asarray(lengths)
    pool_ranges = np.asarray(pool_ranges)
    tk = int(top_k); ly = int(layer); tl = int(total_layers)
    num = tl - ly
    k_arr = np.maximum(tk, (num * lengths.astype(np.int64) + tl - 1) // tl)
    k_arr = np.minimum(k_arr, pool_ranges.astype(np.int64)).astype(np.int64)
    pr = pool_ranges.astype(np.int64)

    assert x.shape == (B, T, C)
    assert (pr >= T // 2).all() and (pr <= T).all()
    assert (pr - k_arr <= MAXDROP).all() and (k_arr <= CHA + CHB).all()

    nc = _get_program()

    iota8 = np.broadcast_to(np.arange(8, dtype=np.float32), (128, 8)).copy()
    ident = np.eye(128, dtype=np.float32)

    in_maps = []
    for core in range(NCORES):
        sl = slice(core * SPC, (core + 1) * SPC)
        prs = pr[sl]; ks = k_arr[sl]
        mrow = np.zeros((SPC, 128, 2048), np.float32)
        kf = np.zeros((SPC, 128, 1), np.float32)
        isl = np.zeros((SPC, 128, 1), np.float32)
        for s in range(SPC):
            L = int(prs[s])
            mrow[s, :, max(L - 2048, 0):] = NEG
            kf[s] = float(ks[s])
            isl[s] = DAMP / (L * PHI0)
        in_maps.append({
            "x2": np.ascontiguousarray(x[sl]),
            "mrowb": mrow,
            "kf": kf,
            "isl": isl,
            "iota8": iota8,
            "ident": ident,
        })

    res = run_bass_kernel_spmd(nc, in_maps, list(range(NCORES)))
    out = np.concatenate([r["y2"] for r in res.results], axis=0)
    return out, k_arr.astype(np.int32)


# revision 14
# speedup vs baseline: 1.0704x; 1.0704x over previous
"""Trainium2 Bass kernel for dynamic k-max pooling (per-column top-k with
order-preserving compaction), data-parallel over batch across 8 NeuronCores.

Self-contained: hardcodes shapes B=16, T=4096, C=256. Host does only O(B)
scalar prep (per-sample k, Newton slope, tail mask rows); all O(B*T*C) work
runs on-device:
  - DMA + PE-transpose x[s] into [C, T] layout; invalid tail rows masked to
    -1e30 during PSUM evacuation
  - per-column exact threshold t* (k-th largest): 4 damped Newton iterations
    on exact is_ge counts, then exact order-statistic extraction through a
    two-sided top-8 window (nc.vector.max on masked/negated streams)
  - output positions via custom-DVE prefix-scan ops
  - order-preserving compaction via gpsimd local_scatter of the two 16-bit
    halves of each f32, then a single shift-or recombine
  - PE-transpose back to [T, C]; DMA out
"""

import numpy as np

import concourse.bass as bass
import concourse.mybir as mybir
import concourse.tile as tile
from concourse import bacc
import concourse.dve_ops as dve_ops_mod
from concourse.dve_ops import DveOp
from concourse.dve_spec import (
    Spec, Src0, C0, C1, C2, Zero, One, AluOp,
    scan, select, lower, _has_src1 as has_src1,
)
from concourse.dve_uop import DveOpSpec
from concourse.bass_utils import run_bass_kernel_spmd

F32 = mybir.dt.float32
U32 = mybir.dt.uint32
I16 = mybir.dt.int16
U16 = mybir.dt.uint16
BF16 = mybir.dt.bfloat16
Alu = mybir.AluOpType
Act = mybir.ActivationFunctionType

B, T, C = 16, 4096, 256
NCORES = 8
SPC = B // NCORES          # samples per core
NEG = np.float32(-1e30)
TH0 = -0.6745              # initial threshold guess (25% drop quantile)
PHI0 = 0.31777657          # N(0,1) pdf at TH0
DAMP = 0.6
SC = float(2.0 ** -20)     # exact pow2 scale for the masked-neg pass
CHA = 2046                 # scatter chunk A covers pos [0, 2046)
CHB = 1026                 # chunk B covers pos [2046, 3072)
WA = 3070                  # chunk A source window [0, 3070)  (2046 + 1024)
MAXDROP = 1024


# ---------------- custom DVE ops ----------------------------------------- #

def _register(name, spec, subdim=False):
    if name in dve_ops_mod._SUB_OPCODE_FOR_NAME:
        return next(op for op in dve_ops_mod.OPS if op.name == name)
    row = dve_ops_mod._CUSTOM_DVE_ROW_BASE + len(dve_ops_mod.OPS)
    assert row < 0x20
    shas = {}
    for ver in ("v3", "v4"):
        uops = lower(spec, ver=ver)
        tmp = DveOpSpec(name=name, opcode=row, uops=uops, rd1_en=has_src1(spec))
        shas[ver] = tmp.sha(ver)
    op = DveOp(name, spec, subdim=subdim, uops_sha=shas)
    dve_ops_mod.OPS.append(op)
    dve_ops_mod._SUB_OPCODE_FOR_NAME[name] = row
    dve_ops_mod.CUSTOM_DVE_SPECS[name] = spec
    return op


def _ref_with_accum(body):
    def r(in0, s0, s1, imm2):
        o = body(in0, s0, s1, imm2)
        return o, o.reshape(o.shape[0], -1).sum(axis=-1, keepdims=True)
    return r


# wa = (v > t) ? -v*imm2 : -1 ; accum = sum(wa)  -> exact count above
MNSA = _register("DKP_MNSA", Spec(
    body=select(Src0 > C0, (Zero - Src0) * C2, Zero - One),
    accum=AluOp.ADD,
    reference=_ref_with_accum(lambda in0, s0, s1, imm2: np.where(
        in0 > s0, -in0 * np.float32(imm2), np.float32(-1.0)).astype(np.float32)),
))

# wb = (v > t) ? -1 : v*imm2   (top8 -> 8 largest below-or-equal t, scaled)
MNSB = _register("DKP_MNSB", Spec(
    body=select(Src0 > C0, Zero - One, Src0 * C2),
    reference=lambda in0, s0, s1, imm2: np.where(
        in0 > s0, np.float32(-1.0), in0 * np.float32(imm2)).astype(np.float32),
))

# posA: o = v >= t ; p = scan(+, o, init=-1); out = (o & (p < s1)) ? p : -1
_o = Src0 >= C0
_p = scan(AluOp.ADD, _o, init=Zero - One)
POSA = _register("DKP_POSA", Spec(
    body=select(_o & (_p < C1), _p, Zero - One),
    reference=lambda in0, s0, s1, imm2: (lambda o, p: np.where(
        o & (p < s1), p, np.float32(-1.0)))(
        in0 >= s0,
        np.cumsum(in0 >= s0, axis=-1, dtype=np.float32) - 1.0
    ).astype(np.float32),
))

# posB: o = v >= t ; p = scan(+, o, init=s1); out = (o & (p < imm2)) ? p : -1
_ob = Src0 >= C0
_pb = scan(AluOp.ADD, _ob, init=C1)
POSB = _register("DKP_POSB", Spec(
    body=select(_ob & (_pb < C2), _pb, Zero - One),
    reference=lambda in0, s0, s1, imm2: (lambda o, p: np.where(
        o & (p < imm2), p, np.float32(-1.0)))(
        in0 >= s0,
        np.cumsum(in0 >= s0, axis=-1, dtype=np.float32) + s1
    ).astype(np.float32),
))


# ---------------- device program ----------------------------------------- #

def build_program():
    import os
    STAGE = int(os.environ.get("DKP_STAGE", "6"))
    nc = bacc.Bacc()
    x2 = nc.declare_dram_parameter("x2", [SPC, T, C], F32, isOutput=False)
    mrowb = nc.declare_dram_parameter("mrowb", [SPC, 128, 2048], F32, isOutput=False)
    kf_d = nc.declare_dram_parameter("kf", [SPC, 128, 1], F32, isOutput=False)
    isl_d = nc.declare_dram_parameter("isl", [SPC, 128, 1], F32, isOutput=False)
    iota8_d = nc.declare_dram_parameter("iota8", [128, 8], F32, isOutput=False)
    ident_d = nc.declare_dram_parameter("ident", [128, 128], F32, isOutput=False)
    y2 = nc.declare_dram_parameter("y2", [SPC, T, C], F32, isOutput=True)

    with tile.TileContext(nc) as tc:
        with (
            tc.tile_pool(name="nat", bufs=1) as nat_pool,
            tc.tile_pool(name="mrow", bufs=2) as mrow_pool,
            tc.tile_pool(name="xt", bufs=2) as xt_pool,
            tc.tile_pool(name="scr", bufs=1) as scr_pool,
            tc.tile_pool(name="w", bufs=1) as w_pool,
            tc.tile_pool(name="pos", bufs=1) as pos_pool,
            tc.tile_pool(name="pl", bufs=1) as pl_pool,
            tc.tile_pool(name="dst", bufs=2) as dst_pool,
            tc.tile_pool(name="ot", bufs=1) as ot_pool,
            tc.tile_pool(name="onat", bufs=1) as onat_pool,
            tc.tile_pool(name="small", bufs=4) as sm_pool,
            tc.tile_pool(name="cst", bufs=1) as cst_pool,
            tc.tile_pool(name="ps", bufs=4, space="PSUM") as ps_pool,
        ):
            ident = cst_pool.tile([128, 128], F32, tag="ident")
            nc.sync.dma_start(ident[:], ident_d[:])
            iota8 = cst_pool.tile([128, 8], F32, tag="iota8")
            nc.sync.dma_start(iota8[:], iota8_d[:])

            for s in range(SPC):
                mrow_t = mrow_pool.tile([128, 2048], F32, tag="mrow")
                nc.sync.dma_start(mrow_t[:], mrowb[s])
                kf_t = sm_pool.tile([128, 1], F32, tag="kf")
                nc.sync.dma_start(kf_t[:], kf_d[s])
                isl_t = sm_pool.tile([128, 1], F32, tag="isl")
                nc.sync.dma_start(isl_t[:], isl_d[s])

                for ct in range(2):
                    # ---- load natural layout [128 rows, 32 blocks, 128 cols]
                    nat = nat_pool.tile([128, 32, 128], F32, tag="nat")
                    src = x2[s].rearrange("(b p) c -> p b c", p=128)
                    nc.sync.dma_start(nat[:], src[:, :, ct * 128:(ct + 1) * 128])

                    # ---- transpose to xT [128 cols, 4096 rows], mask tail
                    xt = xt_pool.tile([128, T], F32, tag="xt")
                    for g in range(8):
                        ps = ps_pool.tile([128, 512], F32, tag="psA")
                        for j in range(4):
                            nc.tensor.transpose(
                                ps[:, j * 128:(j + 1) * 128],
                                nat[:, 4 * g + j, :], ident[:],
                            )
                        if g < 4:
                            nc.scalar.activation(
                                xt[:, g * 512:(g + 1) * 512], ps[:], Act.Copy)
                        else:
                            nc.vector.tensor_tensor(
                                xt[:, g * 512:(g + 1) * 512], ps[:],
                                mrow_t[:, (g - 4) * 512:(g - 3) * 512],
                                Alu.add)

                    # ---- Newton iterations for threshold
                    if STAGE < 2:
                        onat = onat_pool.tile([128, 32, 128], F32, tag="onat")
                        for g in range(8):
                            ps2 = ps_pool.tile([128, 512], F32, tag="psB")
                            for j in range(4):
                                blk = 4 * g + j
                                nc.tensor.transpose(
                                    ps2[:, j * 128:(j + 1) * 128],
                                    xt[:, blk * 128:(blk + 1) * 128], ident[:])
                            nc.scalar.activation(
                                onat[:, 4 * g:4 * g + 4, :].rearrange(
                                    "p b c -> p (b c)"),
                                ps2[:], Act.Copy)
                        dstv = y2[s].rearrange("(b p) c -> p b c", p=128)
                        nc.sync.dma_start(
                            dstv[:, :, ct * 128:(ct + 1) * 128], onat[:])
                        continue
                    th = sm_pool.tile([128, 1], F32, tag="th")
                    nc.vector.memset(th[:], TH0)
                    scr = scr_pool.tile([128, T], BF16, tag="scr")
                    nt = sm_pool.tile([128, 1], F32, tag="nt")
                    d1 = sm_pool.tile([128, 1], F32, tag="d1")
                    for it in range(4):
                        nc.vector.tensor_scalar(
                            scr[:], xt[:], th[:], None, Alu.is_ge,
                            Alu.add, accum_out=nt[:])
                        nc.vector.scalar_tensor_tensor(
                            d1[:], nt[:], kf_t[:], isl_t[:],
                            Alu.subtract, Alu.mult)
                        nc.vector.tensor_tensor(th[:], th[:], d1[:], Alu.add)

                    # ---- two-sided order-statistic window
                    if STAGE < 3:
                        ot = ot_pool.tile([128, T], F32, tag="ot")
                        nc.vector.tensor_scalar(ot[:], xt[:], th[:], None, Alu.subtract)
                        onat = onat_pool.tile([128, 32, 128], F32, tag="onat")
                        for g in range(8):
                            ps2 = ps_pool.tile([128, 512], F32, tag="psB")
                            for j in range(4):
                                blk = 4 * g + j
                                nc.tensor.transpose(
                                    ps2[:, j * 128:(j + 1) * 128],
                                    ot[:, blk * 128:(blk + 1) * 128], ident[:])
                            nc.scalar.activation(
                                onat[:, 4 * g:4 * g + 4, :].rearrange(
                                    "p b c -> p (b c)"),
                                ps2[:], Act.Copy)
                        dstv = y2[s].rearrange("(b p) c -> p b c", p=128)
                        nc.sync.dma_start(
                            dstv[:, :, ct * 128:(ct + 1) * 128], onat[:])
                        continue
                    wa = w_pool.tile([128, T], F32, tag="w")
                    swa = sm_pool.tile([128, 1], F32, tag="swa")
                    nc.vector._custom_dve(
                        MNSA, out=wa[:], in0=xt[:], s0=th[:], imm2=SC,
                        accum_out=swa[:])
                    wa8 = sm_pool.tile([128, 8], F32, tag="wa8")
                    nc.vector.max(wa8[:], wa[:])
                    wb = w_pool.tile([128, T], F32, tag="w")
                    nc.vector._custom_dve(MNSB, out=wb[:], in0=xt[:], s0=th[:],
                                          imm2=SC)
                    wb8 = sm_pool.tile([128, 8], F32, tag="wb8")
                    nc.vector.max(wb8[:], wb[:])

                    # ---- t* = selected order statistic
                    # ma = (T - k) + swa   (= m up to frac error in (-0.024, 0))
                    ma = sm_pool.tile([128, 1], F32, tag="ma")
                    nc.vector.tensor_scalar(
                        ma[:], kf_t[:], -1.0, float(T), Alu.mult, Alu.add)
                    nc.vector.tensor_tensor(ma[:], ma[:], swa[:], Alu.add)
                    da = sm_pool.tile([128, 8], F32, tag="da")
                    nc.vector.tensor_scalar(da[:], iota8[:], ma[:], None,
                                            Alu.subtract)
                    mka = sm_pool.tile([128, 8], F32, tag="mka")
                    nc.vector.tensor_tensor(mka[:], da[:], da[:], Alu.mult)
                    nc.vector.tensor_scalar(mka[:], mka[:], 0.2, None, Alu.is_lt)
                    ta = sm_pool.tile([128, 1], F32, tag="ta")
                    junk8 = sm_pool.tile([128, 8], F32, tag="junk8")
                    nc.vector.tensor_tensor(junk8[:], mka[:], wa8[:], Alu.mult)
                    nc.vector.tensor_reduce(
                        ta[:], junk8[:], mybir.AxisListType.X, Alu.add)
                    # mb = -ma - 1
                    mb = sm_pool.tile([128, 1], F32, tag="mb")
                    nc.vector.tensor_scalar(mb[:], ma[:], -1.0, -1.0,
                                            Alu.mult, Alu.add)
                    db = sm_pool.tile([128, 8], F32, tag="db")
                    nc.vector.tensor_scalar(db[:], iota8[:], mb[:], None,
                                            Alu.subtract)
                    mkb = sm_pool.tile([128, 8], F32, tag="mkb")
                    nc.vector.tensor_tensor(mkb[:], db[:], db[:], Alu.mult)
                    nc.vector.tensor_scalar(mkb[:], mkb[:], 0.2, None, Alu.is_lt)
                    tb = sm_pool.tile([128, 1], F32, tag="tb")
                    junk8b = sm_pool.tile([128, 8], F32, tag="junk8b")
                    nc.vector.tensor_tensor(junk8b[:], mkb[:], wb8[:], Alu.mult)
                    nc.vector.tensor_reduce(
                        tb[:], junk8b[:], mybir.AxisListType.X, Alu.add)
                    tstar = sm_pool.tile([128, 1], F32, tag="tstar")
                    nc.vector.tensor_tensor(tstar[:], tb[:], ta[:], Alu.subtract)
                    nc.vector.tensor_scalar(tstar[:], tstar[:],
                                            float(2.0 ** 20), None, Alu.mult)

                    # ---- positions
                    if STAGE < 4:
                        ot = ot_pool.tile([128, T], F32, tag="ot")
                        nc.vector.tensor_scalar(ot[:], xt[:], tstar[:], None, Alu.is_ge)
                        onat = onat_pool.tile([128, 32, 128], F32, tag="onat")
                        for g in range(8):
                            ps2 = ps_pool.tile([128, 512], F32, tag="psB")
                            for j in range(4):
                                blk = 4 * g + j
                                nc.tensor.transpose(
                                    ps2[:, j * 128:(j + 1) * 128],
                                    ot[:, blk * 128:(blk + 1) * 128], ident[:])
                            nc.scalar.activation(
                                onat[:, 4 * g:4 * g + 4, :].rearrange(
                                    "p b c -> p (b c)"),
                                ps2[:], Act.Copy)
                        dstv = y2[s].rearrange("(b p) c -> p b c", p=128)
                        nc.sync.dma_start(
                            dstv[:, :, ct * 128:(ct + 1) * 128], onat[:])
                        continue
                    nbf = sm_pool.tile([128, 1], F32, tag="nbf")
                    nc.vector.tensor_scalar(
                        scr[:, :CHA], xt[:, :CHA], tstar[:], None, Alu.is_ge,
                        Alu.add, accum_out=nbf[:])
                    pbinit = sm_pool.tile([128, 1], F32, tag="pbinit")
                    nc.vector.tensor_scalar(pbinit[:], nbf[:],
                                            float(-1 - CHA), None, Alu.add)
                    posa = pos_pool.tile([128, WA], I16, tag="posa")
                    nc.vector._custom_dve(
                        POSA, out=posa[:], in0=xt[:, :WA], s0=tstar[:],
                        s1=float(CHA))
                    posb = pos_pool.tile([128, T - CHA], I16, tag="posb")
                    nc.vector._custom_dve(
                        POSB, out=posb[:], in0=xt[:, CHA:], s0=tstar[:],
                        s1=pbinit[:], imm2=float(CHB))

                    # ---- 16-bit planes (little-endian halves, strided copies)
                    xt_h = xt[:].bitcast(U16).rearrange("p (t two) -> p t two", two=2)
                    hi16 = pl_pool.tile([128, T], U16, tag="hi16")
                    nc.vector.tensor_copy(hi16[:], xt_h[:, :, 1])
                    lo16 = pl_pool.tile([128, T], U16, tag="lo16")
                    nc.vector.tensor_copy(lo16[:], xt_h[:, :, 0])

                    # ---- scatters
                    if STAGE < 5:
                        ot = ot_pool.tile([128, T], F32, tag="ot")
                        nc.vector.tensor_copy(ot[:, :WA], posa[:])
                        nc.vector.tensor_copy(ot[:, WA:WA + (T - CHA)], posb[:])
                        nc.vector.memset(ot[:, WA + T - CHA:], 0.0)
                        onat = onat_pool.tile([128, 32, 128], F32, tag="onat")
                        for g in range(8):
                            ps2 = ps_pool.tile([128, 512], F32, tag="psB")
                            for j in range(4):
                                blk = 4 * g + j
                                nc.tensor.transpose(
                                    ps2[:, j * 128:(j + 1) * 128],
                                    ot[:, blk * 128:(blk + 1) * 128], ident[:])
                            nc.scalar.activation(
                                onat[:, 4 * g:4 * g + 4, :].rearrange(
                                    "p b c -> p (b c)"),
                                ps2[:], Act.Copy)
                        dstv = y2[s].rearrange("(b p) c -> p b c", p=128)
                        nc.sync.dma_start(
                            dstv[:, :, ct * 128:(ct + 1) * 128], onat[:])
                        continue
                    dstA_lo = dst_pool.tile([128, CHA], U16, tag="dal")
                    dstA_hi = dst_pool.tile([128, CHA], U16, tag="dah")
                    dstB_lo = dst_pool.tile([128, CHB], U16, tag="dbl")
                    dstB_hi = dst_pool.tile([128, CHB], U16, tag="dbh")
                    nc.gpsimd.local_scatter(
                        dstA_lo[:], lo16[:, :WA], posa[:],
                        channels=128, num_elems=CHA, num_idxs=WA)
                    nc.gpsimd.local_scatter(
                        dstA_hi[:], hi16[:, :WA], posa[:],
                        channels=128, num_elems=CHA, num_idxs=WA)
                    nc.gpsimd.local_scatter(
                        dstB_lo[:], lo16[:, CHA:], posb[:],
                        channels=128, num_elems=CHB, num_idxs=T - CHA)
                    nc.gpsimd.local_scatter(
                        dstB_hi[:], hi16[:, CHA:], posb[:],
                        channels=128, num_elems=CHB, num_idxs=T - CHA)

                    # ---- recombine into outT f32 [128, 4096] (interleaving copies)
                    ot = ot_pool.tile([128, T], F32, tag="ot")
                    ot_h = ot[:].bitcast(U16).rearrange("p (t two) -> p t two", two=2)
                    nc.vector.tensor_copy(ot_h[:, :CHA, 0], dstA_lo[:])
                    nc.vector.tensor_copy(ot_h[:, :CHA, 1], dstA_hi[:])
                    nc.vector.tensor_copy(ot_h[:, CHA:CHA + CHB, 0], dstB_lo[:])
                    nc.vector.tensor_copy(ot_h[:, CHA:CHA + CHB, 1], dstB_hi[:])
                    nc.vector.memset(ot[:, CHA + CHB:], 0.0)

                    # ---- transpose back + store
                    onat = onat_pool.tile([128, 32, 128], F32, tag="onat")
                    for g in range(8):
                        ps2 = ps_pool.tile([128, 512], F32, tag="psB")
                        for j in range(4):
                            blk = 4 * g + j
                            nc.tensor.transpose(
                                ps2[:, j * 128:(j + 1) * 128],
                                ot[:, blk * 128:(blk + 1) * 128], ident[:],
                            )
                        nc.scalar.activation(
                            onat[:, 4 * g:4 * g + 4, :].rearrange(
                                "p b c -> p (b c)"),
                            ps2[:], Act.Copy)
                    dstv = y2[s].rearrange("(b p) c -> p b c", p=128)
                    nc.sync.dma_start(
                        dstv[:, :, ct * 128:(ct + 1) * 128], onat[:])
    return nc


_PROGRAM = None


def _get_program():
    global _PROGRAM
    if _PROGRAM is None:
        nc = build_program()
        if not nc.is_finalized():
            nc.finalize()
        _PROGRAM = nc
    return _PROGRAM


def kernel(x, lengths, pool_ranges, top_k, layer, total_layers):
    x = np.asarray(x, dtype=np.float32)
    lengths = np.asarray(lengths)
    pool_ranges = np.asarray(pool_ranges)
    tk = int(top_k); ly = int(layer); tl = int(total_layers)
    num = tl - ly
    k_arr = np.maximum(tk, (num * lengths.astype(np.int64) + tl - 1) // tl)
    k_arr = np.minimum(k_arr, pool_ranges.astype(np.int64)).astype(np.int64)
    pr = pool_ranges.astype(np.int64)

    assert x.shape == (B, T, C)
    assert (pr >= T // 2).all() and (pr <= T).all()
    assert (pr - k_arr <= MAXDROP).all() and (k_arr <= CHA + CHB).all()

    nc = _get_program()

    iota8 = np.broadcast_to(np.arange(8, dtype=np.float32), (128, 8)).copy()
    ident = np.eye(128, dtype=np.float32)

    in_maps = []
    for core in range(NCORES):
        sl = slice(core * SPC, (core + 1) * SPC)
        prs = pr[sl]; ks = k_arr[sl]
        mrow = np.zeros((SPC, 128, 2048), np.float32)
        kf = np.zeros((SPC, 128, 1), np.float32)
        isl = np.zeros((SPC, 128, 1), np.float32)
        for s in range(SPC):
            L = int(prs[s])
            mrow[s, :, max(L - 2048, 0):] = NEG
            kf[s] = float(ks[s])
            isl[s] = DAMP / (L * PHI0)
        in_maps.append({
            "x2": np.ascontiguousarray(x[sl]),
            "mrowb": mrow,
            "kf": kf,
            "isl": isl,
            "iota8": iota8,
            "ident": ident,
        })

    out = _run_cached(nc, in_maps)
    return out, k_arr.astype(np.int32)


_CACHED = None


def _run_cached(nc, in_maps):
    """Compile the SPMD executable once; reuse across kernel() calls."""
    global _CACHED
    import jax
    from jax.sharding import Mesh, PartitionSpec
    from jax.experimental.shard_map import shard_map
    from concourse import bass2jax, mybir as _mb

    if _CACHED is None:
        bass2jax.install_neuronx_cc_hook()
        partition_name = (
            nc.partition_id_tensor.name if nc.partition_id_tensor else None
        )
        in_names, out_names, out_avals, zero_outs = [], [], [], []
        for alloc in nc.m.functions[0].allocations:
            if not isinstance(alloc, _mb.MemoryLocationSet):
                continue
            name = alloc.memorylocations[0].name
            if alloc.kind == "ExternalInput":
                if name != partition_name:
                    in_names.append(name)
            elif alloc.kind == "ExternalOutput":
                out_names.append(name)
                shape = tuple(alloc.tensor_shape)
                dtype = _mb.dt.np(alloc.dtype)
                out_avals.append(jax.core.ShapedArray(shape, dtype))
                zero_outs.append(np.zeros(shape, dtype))
        n_params = len(in_names)
        all_in_names = in_names + out_names
        if partition_name is not None:
            all_in_names.append(partition_name)

        def _body(*args):
            operands = list(args)
            if partition_name is not None:
                operands.append(bass2jax.partition_id_tensor())
            outs = bass2jax._bass_exec_p.bind(
                *operands,
                out_avals=tuple(out_avals),
                in_names=tuple(all_in_names),
                out_names=tuple(out_names),
                lowering_input_output_aliases=(),
                sim_require_finite=True,
                sim_require_nnan=True,
                nc=nc,
            )
            return tuple(outs)

        devices = jax.devices()[:NCORES]
        mesh = Mesh(np.asarray(devices), ("core",))
        n_outs = len(out_names)
        sharded = jax.jit(
            shard_map(
                _body, mesh=mesh,
                in_specs=(PartitionSpec("core"),) * (n_params + n_outs),
                out_specs=(PartitionSpec("core"),) * n_outs,
                check_rep=False,
            ),
            keep_unused=True,
        )
        _CACHED = (sharded, in_names, out_names, out_avals, zero_outs)

    sharded, in_names, out_names, out_avals, zero_outs = _CACHED
    concat_in = [
        np.concatenate([np.asarray(m[name]) for m in in_maps], axis=0)
        for name in in_names
    ]
    concat_zeros = [
        np.zeros((NCORES * z.shape[0], *z.shape[1:]), z.dtype) for z in zero_outs
    ]
    out_arrs = sharded(*concat_in, *concat_zeros)
    i = out_names.index("y2")
    y = np.asarray(out_arrs[i]).reshape(NCORES * SPC, T, C)
    return y


# revision 18
# speedup vs baseline: 2513.5584x; 2348.3485x over previous
"""Trainium2 Bass kernel for dynamic k-max pooling (per-column top-k with
order-preserving compaction), data-parallel over batch across 8 NeuronCores.

Self-contained: hardcodes shapes B=16, T=4096, C=256. Host does only O(B)
scalar prep (per-sample k, Newton slope, tail mask rows); all O(B*T*C) work
runs on-device:
  - DMA + PE-transpose x[s] into [C, T] layout; invalid tail rows masked to
    -1e30 during PSUM evacuation
  - per-column exact threshold t* (k-th largest): 4 damped Newton iterations
    on exact is_ge counts, then exact order-statistic extraction through a
    two-sided top-8 window (nc.vector.max on masked/negated streams)
  - output positions via custom-DVE prefix-scan ops
  - order-preserving compaction via gpsimd local_scatter of the two 16-bit
    halves of each f32, then a single shift-or recombine
  - PE-transpose back to [T, C]; DMA out
"""

import numpy as np

import concourse.bass as bass
import concourse.mybir as mybir
import concourse.tile as tile
from concourse import bacc
import concourse.dve_ops as dve_ops_mod
from concourse.dve_ops import DveOp
from concourse.dve_spec import (
    Spec, Src0, C0, C1, C2, Zero, One, AluOp,
    scan, select, lower, _has_src1 as has_src1,
)
from concourse.dve_uop import DveOpSpec
from concourse.bass_utils import run_bass_kernel_spmd

F32 = mybir.dt.float32
U32 = mybir.dt.uint32
I16 = mybir.dt.int16
U16 = mybir.dt.uint16
BF16 = mybir.dt.bfloat16
Alu = mybir.AluOpType
Act = mybir.ActivationFunctionType

B, T, C = 16, 4096, 256
NCORES = 8
SPC = B // NCORES          # samples per core
NEG = np.float32(-1e30)
TH0 = -0.6745              # initial threshold guess (25% drop quantile)
PHI0 = 0.31777657          # N(0,1) pdf at TH0
DAMP = 0.6
SC = float(2.0 ** -20)     # exact pow2 scale for the masked-neg pass
CHA = 2046                 # scatter chunk A covers pos [0, 2046)
CHB = 1026                 # chunk B covers pos [2046, 3072)
WA = 3070                  # chunk A source window [0, 3070)  (2046 + 1024)
MAXDROP = 1024


# ---------------- custom DVE ops ----------------------------------------- #

def _register(name, spec, subdim=False):
    if name in dve_ops_mod._SUB_OPCODE_FOR_NAME:
        return next(op for op in dve_ops_mod.OPS if op.name == name)
    row = dve_ops_mod._CUSTOM_DVE_ROW_BASE + len(dve_ops_mod.OPS)
    assert row < 0x20
    shas = {}
    for ver in ("v3", "v4"):
        uops = lower(spec, ver=ver)
        tmp = DveOpSpec(name=name, opcode=row, uops=uops, rd1_en=has_src1(spec))
        shas[ver] = tmp.sha(ver)
    op = DveOp(name, spec, subdim=subdim, uops_sha=shas)
    dve_ops_mod.OPS.append(op)
    dve_ops_mod._SUB_OPCODE_FOR_NAME[name] = row
    dve_ops_mod.CUSTOM_DVE_SPECS[name] = spec
    return op


def _ref_with_accum(body):
    def r(in0, s0, s1, imm2):
        o = body(in0, s0, s1, imm2)
        return o, o.reshape(o.shape[0], -1).sum(axis=-1, keepdims=True)
    return r


# wa = (v > t) ? -v*imm2 : -1 ; accum = sum(wa)  -> exact count above
MNSA = _register("DKP_MNSA", Spec(
    body=select(Src0 > C0, (Zero - Src0) * C2, Zero - One),
    accum=AluOp.ADD,
    reference=_ref_with_accum(lambda in0, s0, s1, imm2: np.where(
        in0 > s0, -in0 * np.float32(imm2), np.float32(-1.0)).astype(np.float32)),
))

# wb = (v > t) ? -1 : v*imm2   (top8 -> 8 largest below-or-equal t, scaled)
MNSB = _register("DKP_MNSB", Spec(
    body=select(Src0 > C0, Zero - One, Src0 * C2),
    reference=lambda in0, s0, s1, imm2: np.where(
        in0 > s0, np.float32(-1.0), in0 * np.float32(imm2)).astype(np.float32),
))

# posA: o = v >= t ; p = scan(+, o, init=-1); out = (o & (p < s1)) ? p : -1
_o = Src0 >= C0
_p = scan(AluOp.ADD, _o, init=Zero - One)
POSA = _register("DKP_POSA", Spec(
    body=select(_o & (_p < C1), _p, Zero - One),
    reference=lambda in0, s0, s1, imm2: (lambda o, p: np.where(
        o & (p < s1), p, np.float32(-1.0)))(
        in0 >= s0,
        np.cumsum(in0 >= s0, axis=-1, dtype=np.float32) - 1.0
    ).astype(np.float32),
))

# posB: o = v >= t ; p = scan(+, o, init=s1); out = (o & (p < imm2)) ? p : -1
_ob = Src0 >= C0
_pb = scan(AluOp.ADD, _ob, init=C1)
POSB = _register("DKP_POSB", Spec(
    body=select(_ob & (_pb < C2), _pb, Zero - One),
    reference=lambda in0, s0, s1, imm2: (lambda o, p: np.where(
        o & (p < imm2), p, np.float32(-1.0)))(
        in0 >= s0,
        np.cumsum(in0 >= s0, axis=-1, dtype=np.float32) + s1
    ).astype(np.float32),
))


# ---------------- device program ----------------------------------------- #

def build_program():
    import os
    STAGE = int(os.environ.get("DKP_STAGE", "6"))
    REPS = int(os.environ.get("DKP_REPS", "1"))
    nc = bacc.Bacc()
    x2 = nc.declare_dram_parameter("x2", [SPC, T, C], F32, isOutput=False)
    mrowb = nc.declare_dram_parameter("mrowb", [SPC, 128, 2048], F32, isOutput=False)
    kf_d = nc.declare_dram_parameter("kf", [SPC, 128, 1], F32, isOutput=False)
    isl_d = nc.declare_dram_parameter("isl", [SPC, 128, 1], F32, isOutput=False)
    iota8_d = nc.declare_dram_parameter("iota8", [128, 8], F32, isOutput=False)
    ident_d = nc.declare_dram_parameter("ident", [128, 128], F32, isOutput=False)
    y2 = nc.declare_dram_parameter("y2", [SPC, T, C], F32, isOutput=True)

    with tile.TileContext(nc) as tc:
        with (
            tc.tile_pool(name="nat", bufs=1) as nat_pool,
            tc.tile_pool(name="mrow", bufs=2) as mrow_pool,
            tc.tile_pool(name="xt", bufs=2) as xt_pool,
            tc.tile_pool(name="scr", bufs=1) as scr_pool,
            tc.tile_pool(name="w", bufs=1) as w_pool,
            tc.tile_pool(name="pos", bufs=2) as pos_pool,
            tc.tile_pool(name="pl", bufs=2) as pl_pool,
            tc.tile_pool(name="dst", bufs=1) as dst_pool,
            tc.tile_pool(name="ot", bufs=1) as ot_pool,
            tc.tile_pool(name="onat", bufs=1) as onat_pool,
            tc.tile_pool(name="small", bufs=4) as sm_pool,
            tc.tile_pool(name="cst", bufs=1) as cst_pool,
            tc.tile_pool(name="ps", bufs=4, space="PSUM") as ps_pool,
        ):
            ident = cst_pool.tile([128, 128], F32, tag="ident")
            nc.sync.dma_start(ident[:], ident_d[:])
            iota8 = cst_pool.tile([128, 8], F32, tag="iota8")
            nc.sync.dma_start(iota8[:], iota8_d[:])

            for rep in range(REPS):
              for s in range(SPC):
                mrow_t = mrow_pool.tile([128, 2048], F32, tag="mrow")
                nc.sync.dma_start(mrow_t[:], mrowb[s])
                kf_t = sm_pool.tile([128, 1], F32, tag="kf")
                nc.sync.dma_start(kf_t[:], kf_d[s])
                isl_t = sm_pool.tile([128, 1], F32, tag="isl")
                nc.sync.dma_start(isl_t[:], isl_d[s])

                for ct in range(2):
                    # ---- load natural layout [128 rows, 32 blocks, 128 cols]
                    nat = nat_pool.tile([128, 32, 128], F32, tag="nat")
                    src = x2[s].rearrange("(b p) c -> p b c", p=128)
                    nc.sync.dma_start(nat[:], src[:, :, ct * 128:(ct + 1) * 128])

                    # ---- transpose to xT [128 cols, 4096 rows], mask tail
                    xt = xt_pool.tile([128, T], F32, tag="xt")
                    for g in range(8):
                        ps = ps_pool.tile([128, 512], F32, tag="psA")
                        for j in range(4):
                            nc.tensor.transpose(
                                ps[:, j * 128:(j + 1) * 128],
                                nat[:, 4 * g + j, :], ident[:],
                            )
                        if g < 4:
                            nc.scalar.activation(
                                xt[:, g * 512:(g + 1) * 512], ps[:], Act.Copy)
                        else:
                            nc.vector.tensor_tensor(
                                xt[:, g * 512:(g + 1) * 512], ps[:],
                                mrow_t[:, (g - 4) * 512:(g - 3) * 512],
                                Alu.add)

                    # ---- Newton iterations for threshold
                    if STAGE < 2:
                        onat = onat_pool.tile([128, 32, 128], F32, tag="onat")
                        for g in range(8):
                            ps2 = ps_pool.tile([128, 512], F32, tag="psB")
                            for j in range(4):
                                blk = 4 * g + j
                                nc.tensor.transpose(
                                    ps2[:, j * 128:(j + 1) * 128],
                                    xt[:, blk * 128:(blk + 1) * 128], ident[:])
                            nc.scalar.activation(
                                onat[:, 4 * g:4 * g + 4, :].rearrange(
                                    "p b c -> p (b c)"),
                                ps2[:], Act.Copy)
                        dstv = y2[s].rearrange("(b p) c -> p b c", p=128)
                        nc.sync.dma_start(
                            dstv[:, :, ct * 128:(ct + 1) * 128], onat[:])
                        continue
                    th = sm_pool.tile([128, 1], F32, tag="th")
                    nc.vector.memset(th[:], TH0)
                    scr = scr_pool.tile([128, T], BF16, tag="scr")
                    nt = sm_pool.tile([128, 1], F32, tag="nt")
                    d1 = sm_pool.tile([128, 1], F32, tag="d1")
                    for it in range(4):
                        nc.vector.tensor_scalar(
                            scr[:], xt[:], th[:], None, Alu.is_ge,
                            Alu.add, accum_out=nt[:])
                        nc.vector.scalar_tensor_tensor(
                            d1[:], nt[:], kf_t[:], isl_t[:],
                            Alu.subtract, Alu.mult)
                        nc.vector.tensor_tensor(th[:], th[:], d1[:], Alu.add)

                    # ---- two-sided order-statistic window
                    if STAGE < 3:
                        ot = ot_pool.tile([128, T], F32, tag="ot")
                        nc.vector.tensor_scalar(ot[:], xt[:], th[:], None, Alu.subtract)
                        onat = onat_pool.tile([128, 32, 128], F32, tag="onat")
                        for g in range(8):
                            ps2 = ps_pool.tile([128, 512], F32, tag="psB")
                            for j in range(4):
                                blk = 4 * g + j
                                nc.tensor.transpose(
                                    ps2[:, j * 128:(j + 1) * 128],
                                    ot[:, blk * 128:(blk + 1) * 128], ident[:])
                            nc.scalar.activation(
                                onat[:, 4 * g:4 * g + 4, :].rearrange(
                                    "p b c -> p (b c)"),
                                ps2[:], Act.Copy)
                        dstv = y2[s].rearrange("(b p) c -> p b c", p=128)
                        nc.sync.dma_start(
                            dstv[:, :, ct * 128:(ct + 1) * 128], onat[:])
                        continue
                    wa = w_pool.tile([128, T], F32, tag="w")
                    swa = sm_pool.tile([128, 1], F32, tag="swa")
                    nc.vector._custom_dve(
                        MNSA, out=wa[:], in0=xt[:], s0=th[:], imm2=SC,
                        accum_out=swa[:])
                    wa8 = sm_pool.tile([128, 8], F32, tag="wa8")
                    nc.vector.max(wa8[:], wa[:])
                    wb = w_pool.tile([128, T], F32, tag="w")
                    nc.vector._custom_dve(MNSB, out=wb[:], in0=xt[:], s0=th[:],
                                          imm2=SC)
                    wb8 = sm_pool.tile([128, 8], F32, tag="wb8")
                    nc.vector.max(wb8[:], wb[:])

                    # ---- t* = selected order statistic
                    # ma = (T - k) + swa   (= m up to frac error in (-0.024, 0))
                    ma = sm_pool.tile([128, 1], F32, tag="ma")
                    nc.vector.tensor_scalar(
                        ma[:], kf_t[:], -1.0, float(T), Alu.mult, Alu.add)
                    nc.vector.tensor_tensor(ma[:], ma[:], swa[:], Alu.add)
                    da = sm_pool.tile([128, 8], F32, tag="da")
                    nc.vector.tensor_scalar(da[:], iota8[:], ma[:], None,
                                            Alu.subtract)
                    mka = sm_pool.tile([128, 8], F32, tag="mka")
                    nc.vector.tensor_tensor(mka[:], da[:], da[:], Alu.mult)
                    nc.vector.tensor_scalar(mka[:], mka[:], 0.2, None, Alu.is_lt)
                    ta = sm_pool.tile([128, 1], F32, tag="ta")
                    junk8 = sm_pool.tile([128, 8], F32, tag="junk8")
                    nc.vector.tensor_tensor(junk8[:], mka[:], wa8[:], Alu.mult)
                    nc.vector.tensor_reduce(
                        ta[:], junk8[:], mybir.AxisListType.X, Alu.add)
                    # mb = -ma - 1
                    mb = sm_pool.tile([128, 1], F32, tag="mb")
                    nc.vector.tensor_scalar(mb[:], ma[:], -1.0, -1.0,
                                            Alu.mult, Alu.add)
                    db = sm_pool.tile([128, 8], F32, tag="db")
                    nc.vector.tensor_scalar(db[:], iota8[:], mb[:], None,
                                            Alu.subtract)
                    mkb = sm_pool.tile([128, 8], F32, tag="mkb")
                    nc.vector.tensor_tensor(mkb[:], db[:], db[:], Alu.mult)
                    nc.vector.tensor_scalar(mkb[:], mkb[:], 0.2, None, Alu.is_lt)
                    tb = sm_pool.tile([128, 1], F32, tag="tb")
                    junk8b = sm_pool.tile([128, 8], F32, tag="junk8b")
                    nc.vector.tensor_tensor(junk8b[:], mkb[:], wb8[:], Alu.mult)
                    nc.vector.tensor_reduce(
                        tb[:], junk8b[:], mybir.AxisListType.X, Alu.add)
                    tstar = sm_pool.tile([128, 1], F32, tag="tstar")
                    nc.vector.tensor_tensor(tstar[:], tb[:], ta[:], Alu.subtract)
                    nc.vector.tensor_scalar(tstar[:], tstar[:],
                                            float(2.0 ** 20), None, Alu.mult)

                    # ---- positions
                    if STAGE < 4:
                        ot = ot_pool.tile([128, T], F32, tag="ot")
                        nc.vector.tensor_scalar(ot[:], xt[:], tstar[:], None, Alu.is_ge)
                        onat = onat_pool.tile([128, 32, 128], F32, tag="onat")
                        for g in range(8):
                            ps2 = ps_pool.tile([128, 512], F32, tag="psB")
                            for j in range(4):
                                blk = 4 * g + j
                                nc.tensor.transpose(
                                    ps2[:, j * 128:(j + 1) * 128],
                                    ot[:, blk * 128:(blk + 1) * 128], ident[:])
                            nc.scalar.activation(
                                onat[:, 4 * g:4 * g + 4, :].rearrange(
                                    "p b c -> p (b c)"),
                                ps2[:], Act.Copy)
                        dstv = y2[s].rearrange("(b p) c -> p b c", p=128)
                        nc.sync.dma_start(
                            dstv[:, :, ct * 128:(ct + 1) * 128], onat[:])
                        continue
                    nbf = sm_pool.tile([128, 1], F32, tag="nbf")
                    nc.vector.tensor_scalar(
                        scr[:, :CHA], xt[:, :CHA], tstar[:], None, Alu.is_ge,
                        Alu.add, accum_out=nbf[:])
                    pbinit = sm_pool.tile([128, 1], F32, tag="pbinit")
                    nc.vector.tensor_scalar(pbinit[:], nbf[:],
                                            float(-1 - CHA), None, Alu.add)
                    posa = pos_pool.tile([128, WA], I16, tag="posa")
                    nc.vector._custom_dve(
                        POSA, out=posa[:], in0=xt[:, :WA], s0=tstar[:],
                        s1=float(CHA))
                    posb = pos_pool.tile([128, T - CHA], I16, tag="posb")
                    nc.vector._custom_dve(
                        POSB, out=posb[:], in0=xt[:, CHA:], s0=tstar[:],
                        s1=pbinit[:], imm2=float(CHB))

                    # ---- 16-bit planes (little-endian halves, strided copies)
                    xt_h = xt[:].bitcast(U16).rearrange("p (t two) -> p t two", two=2)
                    hi16 = pl_pool.tile([128, T], U16, tag="hi16")
                    nc.vector.tensor_copy(hi16[:], xt_h[:, :, 1])
                    lo16 = pl_pool.tile([128, T], U16, tag="lo16")
                    nc.vector.tensor_copy(lo16[:], xt_h[:, :, 0])

                    # ---- scatters
                    if STAGE < 5:
                        ot = ot_pool.tile([128, T], F32, tag="ot")
                        nc.vector.tensor_copy(ot[:, :WA], posa[:])
                        nc.vector.tensor_copy(ot[:, WA:WA + (T - CHA)], posb[:])
                        nc.vector.memset(ot[:, WA + T - CHA:], 0.0)
                        onat = onat_pool.tile([128, 32, 128], F32, tag="onat")
                        for g in range(8):
                            ps2 = ps_pool.tile([128, 512], F32, tag="psB")
                            for j in range(4):
                                blk = 4 * g + j
                                nc.tensor.transpose(
                                    ps2[:, j * 128:(j + 1) * 128],
                                    ot[:, blk * 128:(blk + 1) * 128], ident[:])
                            nc.scalar.activation(
                                onat[:, 4 * g:4 * g + 4, :].rearrange(
                                    "p b c -> p (b c)"),
                                ps2[:], Act.Copy)
                        dstv = y2[s].rearrange("(b p) c -> p b c", p=128)
                        nc.sync.dma_start(
                            dstv[:, :, ct * 128:(ct + 1) * 128], onat[:])
                        continue
                    dstA_lo = dst_pool.tile([128, CHA], U16, tag="dal")
                    dstA_hi = dst_pool.tile([128, CHA], U16, tag="dah")
                    dstB_lo = dst_pool.tile([128, CHB], U16, tag="dbl")
                    dstB_hi = dst_pool.tile([128, CHB], U16, tag="dbh")
                    nc.gpsimd.local_scatter(
                        dstA_lo[:], lo16[:, :WA], posa[:],
                        channels=128, num_elems=CHA, num_idxs=WA)
                    nc.gpsimd.local_scatter(
                        dstA_hi[:], hi16[:, :WA], posa[:],
                        channels=128, num_elems=CHA, num_idxs=WA)
                    nc.gpsimd.local_scatter(
                        dstB_lo[:], lo16[:, CHA:], posb[:],
                        channels=128, num_elems=CHB, num_idxs=T - CHA)
                    nc.gpsimd.local_scatter(
                        dstB_hi[:], hi16[:, CHA:], posb[:],
                        channels=128, num_elems=CHB, num_idxs=T - CHA)

                    # ---- recombine into outT f32 [128, 4096] (interleaving copies)
                    ot = ot_pool.tile([128, T], F32, tag="ot")
                    ot_h = ot[:].bitcast(U16).rearrange("p (t two) -> p t two", two=2)
                    nc.vector.tensor_copy(ot_h[:, :CHA, 0], dstA_lo[:])
                    nc.vector.tensor_copy(ot_h[:, :CHA, 1], dstA_hi[:])
                    nc.vector.tensor_copy(ot_h[:, CHA:CHA + CHB, 0], dstB_lo[:])
                    nc.vector.tensor_copy(ot_h[:, CHA:CHA + CHB, 1], dstB_hi[:])

                    # ---- transpose back + store (tail rows >= 3072 always 0)
                    onat = onat_pool.tile([128, 32, 128], F32, tag="onat")
                    for g in range(6):
                        ps2 = ps_pool.tile([128, 512], F32, tag="psB")
                        for j in range(4):
                            blk = 4 * g + j
                            nc.tensor.transpose(
                                ps2[:, j * 128:(j + 1) * 128],
                                ot[:, blk * 128:(blk + 1) * 128], ident[:],
                            )
                        nc.scalar.activation(
                            onat[:, 4 * g:4 * g + 4, :].rearrange(
                                "p b c -> p (b c)"),
                            ps2[:], Act.Copy)
                    nc.vector.memset(
                        onat[:, 24:, :].rearrange("p b c -> p (b c)"), 0.0)
                    dstv = y2[s].rearrange("(b p) c -> p b c", p=128)
                    nc.sync.dma_start(
                        dstv[:, :, ct * 128:(ct + 1) * 128], onat[:])
    return nc


_PROGRAM = None


def _get_program():
    global _PROGRAM
    if _PROGRAM is None:
        nc = build_program()
        if not nc.is_finalized():
            nc.finalize()
        _PROGRAM = nc
    return _PROGRAM


def kernel(x, lengths, pool_ranges, top_k, layer, total_layers):
    x = np.asarray(x, dtype=np.float32)
    lengths = np.asarray(lengths)
    pool_ranges = np.asarray(pool_ranges)
    tk = int(top_k); ly = int(layer); tl = int(total_layers)
    num = tl - ly
    k_arr = np.maximum(tk, (num * lengths.astype(np.int64) + tl - 1) // tl)
    k_arr = np.minimum(k_arr, pool_ranges.astype(np.int64)).astype(np.int64)
    pr = pool_ranges.astype(np.int64)

    assert x.shape == (B, T, C)
    assert (pr >= T // 2).all() and (pr <= T).all()
    assert (pr - k_arr <= MAXDROP).all() and (k_arr <= CHA + CHB).all()

    nc = _get_program()

    iota8 = np.broadcast_to(np.arange(8, dtype=np.float32), (128, 8)).copy()
    ident = np.eye(128, dtype=np.float32)

    in_maps = []
    for core in range(NCORES):
        sl = slice(core * SPC, (core + 1) * SPC)
        prs = pr[sl]; ks = k_arr[sl]
        mrow = np.zeros((SPC, 128, 2048), np.float32)
        kf = np.zeros((SPC, 128, 1), np.float32)
        isl = np.zeros((SPC, 128, 1), np.float32)
        for s in range(SPC):
            L = int(prs[s])
            mrow[s, :, max(L - 2048, 0):] = NEG
            kf[s] = float(ks[s])
            isl[s] = DAMP / (L * PHI0)
        in_maps.append({
            "x2": np.ascontiguousarray(x[sl]),
            "mrowb": mrow,
            "kf": kf,
            "isl": isl,
            "iota8": iota8,
            "ident": ident,
        })

    out = _run_cached(nc, in_maps)
    return out, k_arr.astype(np.int32)


_CACHED = None


def _run_cached(nc, in_maps):
    """Compile the SPMD executable once; reuse across kernel() calls."""
    global _CACHED
    import jax
    from jax.sharding import Mesh, PartitionSpec
    from jax.experimental.shard_map import shard_map
    from concourse import bass2jax, mybir as _mb

    if _CACHED is None:
        bass2jax.install_neuronx_cc_hook()
        partition_name = (
            nc.partition_id_tensor.name if nc.partition_id_tensor else None
        )
        in_names, out_names, out_avals, zero_outs = [], [], [], []
        for alloc in nc.m.functions[0].allocations:
            if not isinstance(alloc, _mb.MemoryLocationSet):
                continue
            name = alloc.memorylocations[0].name
            if alloc.kind == "ExternalInput":
                if name != partition_name:
                    in_names.append(name)
            elif alloc.kind == "ExternalOutput":
                out_names.append(name)
                shape = tuple(alloc.tensor_shape)
                dtype = _mb.dt.np(alloc.dtype)
                out_avals.append(jax.core.ShapedArray(shape, dtype))
                zero_outs.append(np.zeros(shape, dtype))
        n_params = len(in_names)
        all_in_names = in_names + out_names
        if partition_name is not None:
            all_in_names.append(partition_name)

        def _body(*args):
            operands = list(args)
            if partition_name is not None:
                operands.append(bass2jax.partition_id_tensor())
            outs = bass2jax._bass_exec_p.bind(
                *operands,
                out_avals=tuple(out_avals),
                in_names=tuple(all_in_names),
                out_names=tuple(out_names),
                lowering_input_output_aliases=(),
                sim_require_finite=True,
                sim_require_nnan=True,
                nc=nc,
            )
            return tuple(outs)

        devices = jax.devices()[:NCORES]
        mesh = Mesh(np.asarray(devices), ("core",))
        n_outs = len(out_names)
        sharded = jax.jit(
            shard_map(
                _body, mesh=mesh,
                in_specs=(PartitionSpec("core"),) * (n_params + n_outs),
                out_specs=(PartitionSpec("core"),) * n_outs,
                check_rep=False,
            ),
            keep_unused=True,
        )
        _CACHED = (sharded, in_names, out_names, out_avals, zero_outs)

    sharded, in_names, out_names, out_avals, zero_outs = _CACHED
    concat_in = [
        np.concatenate([np.asarray(m[name]) for m in in_maps], axis=0)
        for name in in_names
    ]
    concat_zeros = [
        np.zeros((NCORES * z.shape[0], *z.shape[1:]), z.dtype) for z in zero_outs
    ]
    out_arrs = sharded(*concat_in, *concat_zeros)
    i = out_names.index("y2")
    y = np.asarray(out_arrs[i]).reshape(NCORES * SPC, T, C)
    return y
